# revision 26
# baseline (speedup 1.0000x reference)
"""Trainium2 Bass kernel for nn_DeepQNetIVCML (GNN message passing).

Reference computation per (b, a) pair:
  multi-hop coverage over a sparse binary adjacency (3 steps), weighted
  feature aggregation, mask + mean-normalize, then a small shared MLP.

Sharding: 128 (b, a) pairs split across 8 cores (16 pairs each; every
core sees exactly one b). MLP weights are replicated.

Key kernel ideas:
  - Propagation runs in "path count" space: p_{t+1} = A^T p_t with no
    thresholding between steps (support(prefix_sum) is exact), so
    cover_t = min(prefix_sum, 1) and the per-node weight is a telescoped
    linear combination of covers.
  - Adjacency and seed vectors are binary -> exact in fp8 e4m3.
    A-stationary matmuls keep the state in column layout.
  - fea = F^T w with F (fea_emb) streamed ONCE in bf16 (the 2e-2
    rel-err budget dwarfs bf16's ~4e-3), computed with the F tile as
    the matmul stationary operand so each matmul has output free size
    1 -> near-zero PE engine time and the result lands directly in
    transposed (column) layout; no transposes anywhere.
  - The per-node weights divided by ALPHA^4 are exact dyadic rationals
    (ALPHA = 0.8 -> {1.953125, 1.5625, 1.25, 1}), exact in bf16; ALPHA^4
    folds into the per-pair scalar.
  - mask/denominator/ALPHA^4 fold into one per-pair scalar, broadcast
    to 128 partitions with a 1-row matmul and applied as the activation
    scale at the relu.
  - MLP weights, query features and all MLP activations are bf16
    (halves their DMA bytes and 4x's the PE matmul rate vs f32).
  - DMA order: s0/mask ride the ACT ring first; the 16 pairs' A/F
    tiles stream on the SP ring with the MLP weight loads interleaved
    mid-stream so they neither delay the first pairs nor gate the tail.
"""

import os
import sys

for _p in ("/opt/trn_rl_repo", "/opt/pypackages"):
    if os.path.isdir(_p) and _p not in sys.path:
        sys.path.insert(0, _p)

import ml_dtypes
import numpy as np

import concourse.bacc as bacc
import concourse.mybir as mybir
from concourse.tile import TileContext

B, A, N, D, L = 4, 32, 512, 768, 128
ALPHA = 0.8
STEP_NUM = 3
NCORES = 8
P_PER = (B * A) // NCORES  # pairs per core
NCH = N // 128             # node chunks
DG = D // 128              # feature chunks

BF16 = mybir.dt.bfloat16
F8 = mybir.dt.float8e4
U8 = mybir.dt.uint8
F32 = mybir.dt.float32
BF16_NP = ml_dtypes.bfloat16
F8_NP = ml_dtypes.float8_e4m3

_PROG = None
LAST_RESULT = None


def _build():
    nc = bacc.Bacc("TRN2", target_bir_lowering=False, debug=False,
                   num_devices=NCORES)

    # adjacency bit-packed 4 entries/byte: 4x less DMA traffic, unpacked
    # on-chip by DVE shift+mask ops (DVE has headroom; DMA is the
    # bottleneck)
    a_pre = nc.dram_tensor("a_pre", [128, P_PER * NCH * N // 4], U8,
                           kind="ExternalInput")
    f_pre = nc.dram_tensor("f_pre", [128, P_PER * NCH * D], BF16,
                           kind="ExternalInput")
    s0_pre = nc.dram_tensor("s0_pre", [128, P_PER * NCH], F8,
                            kind="ExternalInput")
    mask_pre = nc.dram_tensor("mask_pre", [1, P_PER], F32,
                              kind="ExternalInput")
    q_pre = nc.dram_tensor("q_pre", [L, D], BF16, kind="ExternalInput")
    w1_pre = nc.dram_tensor("w1_pre", [128, DG * D], BF16,
                            kind="ExternalInput")
    w2_pre = nc.dram_tensor("w2_pre", [128, 2 * DG * D], BF16,
                            kind="ExternalInput")
    w3_pre = nc.dram_tensor("w3_pre", [128, DG], BF16, kind="ExternalInput")
    b1_pre = nc.dram_tensor("b1_pre", [1, D], BF16, kind="ExternalInput")
    b2_pre = nc.dram_tensor("b2_pre", [1, D], F32, kind="ExternalInput")
    b3_pre = nc.dram_tensor("b3_pre", [1, 1], BF16, kind="ExternalInput")
    y_out = nc.dram_tensor("y", [P_PER, 1], F32, kind="ExternalOutput")

    mult = mybir.AluOpType.mult
    add = mybir.AluOpType.add
    relu = mybir.ActivationFunctionType.Relu

    # per-cover weights scaled by ALPHA^-4: exact dyadic rationals
    c_init = 1.0 / ALPHA**3 - 1.0 / ALPHA**2       # 0.390625
    coefs = [1.0 / ALPHA**2 - 1.0 / ALPHA,         # 0.3125
             1.0 / ALPHA - 1.0,                    # 0.25
             1.0]
    a4 = float(np.float32(ALPHA) ** 4)

    with TileContext(nc) as tc:
        with (
            tc.tile_pool(name="const", bufs=1) as cpool,
            tc.tile_pool(name="weights", bufs=1) as wpool,
            tc.tile_pool(name="abuf", bufs=16) as apool,
            tc.tile_pool(name="aubuf", bufs=6) as aupool,
            tc.tile_pool(name="fbuf", bufs=16) as fpool,
            tc.tile_pool(name="small", bufs=16) as spool,
            tc.tile_pool(name="mlp", bufs=1) as mpool,
        ):
            ones16 = cpool.tile([1, P_PER], BF16)
            nc.vector.memset(ones16[:], 1.0)
            onesL = cpool.tile([128, 1], BF16)
            nc.vector.memset(onesL[:], 1.0 / L)
            ones128 = cpool.tile([128, 1], F32)
            nc.vector.memset(ones128[:], 1.0)
            onesrow = cpool.tile([1, 128], F32)
            nc.vector.memset(onesrow[:], 1.0)

            # the SP HWDGE ring is FIFO. All 16 adjacency tiles go first
            # (11.6us) so every pair's propagation/coverage chain
            # completes mid-stream; then the feature tiles stream with
            # the MLP weight loads interleaved so they neither delay the
            # first features nor arrive after the pair stream ends. The
            # tail after the last F tile is then just fea + MLP.
            NP4 = NCH * N // 4
            staged = {}
            for p in range(P_PER):
                A_sb = apool.tile([128, NP4], U8, tag="A")
                nc.sync.dma_start(A_sb[:], a_pre[:, p * NP4:(p + 1) * NP4])
                staged[p] = [A_sb, None]

            # tiny state/mask loads ride right behind the A block (the
            # propagation that needs them has tens of us of slack, and
            # keeping them off the ACT ring avoids extra HWDGE slots at
            # the head of the stream)
            s0_sb = cpool.tile([128, P_PER * NCH], F8)
            nc.sync.dma_start(s0_sb[:], s0_pre[:])
            mask_sb = cpool.tile([1, P_PER], F32)
            nc.sync.dma_start(mask_sb[:], mask_pre[:])

            def f_dma(p):
                F_sb = fpool.tile([128, NCH * D], BF16, tag="F")
                nc.sync.dma_start(F_sb[:],
                                  f_pre[:, p * NCH * D:(p + 1) * NCH * D])
                staged[p][1] = F_sb

            # q goes first among the F-stream inserts: the q_block's qT
            # copies sit on the ACT queue ahead of later nfT
            # activations, so a late q load would stall the activation
            # queue and with it the fea psum rotation.
            q_sb = cpool.tile([L, D], BF16)
            nc.sync.dma_start(q_sb[:], q_pre[:])
            f_dma(0)
            f_dma(1)
            w3_sb = wpool.tile([128, DG], BF16)
            nc.sync.dma_start(w3_sb[:], w3_pre[:])
            b1row = cpool.tile([1, D], BF16)
            nc.sync.dma_start(b1row[:], b1_pre[:])
            b2row = cpool.tile([1, D], F32)
            nc.sync.dma_start(b2row[:], b2_pre[:])
            b3_sb = cpool.tile([1, 1], BF16)
            nc.sync.dma_start(b3_sb[:], b3_pre[:])
            for p in range(2, 8):
                f_dma(p)
            w2_sb = wpool.tile([128, 2 * DG * D], BF16)
            nc.sync.dma_start(w2_sb[:], w2_pre[:])
            f_dma(8)
            f_dma(9)
            w1_sb = wpool.tile([128, DG * D], BF16)
            nc.sync.dma_start(w1_sb[:], w1_pre[:])
            for p in range(10, P_PER):
                f_dma(p)

            nfT = mpool.tile([128, DG * P_PER], BF16)

            with (
                tc.tile_pool(name="ppps", bufs=2, space="PSUM") as pp_psum,
                tc.tile_pool(name="feaps", bufs=2, space="PSUM") as fea_psum,
                tc.tile_pool(name="denps", bufs=1, space="PSUM") as den_psum,
                tc.tile_pool(name="qps", bufs=1, space="PSUM") as qpsum,
            ):
                # combined h2 bias row: W2q^T qbar + b2, as a [1, D] row
                # so the MLP can fold it in as one bias matmul per group
                qb2r = mpool.tile([1, D], BF16)

                def q_block():
                    # q-side of the MLP: placed mid-loop in PE program
                    # order so its weight-DMA waits never head-block the
                    # PE instruction FIFO. Weight-stationary orientation
                    # produces the transposed column layout directly.
                    qT = mpool.tile([128, DG], BF16)
                    qtp = qpsum.tile([128, 1], F32, tag="qt")
                    for g in range(DG):
                        nc.tensor.matmul(qtp[:],
                                         q_sb[:, g * 128:(g + 1) * 128],
                                         onesL[:], start=True, stop=True)
                        nc.scalar.copy(qT[:, g:g + 1], qtp[:])
                    q2p = qpsum.tile([1, D], F32, tag="q2")
                    for lo, hi in ((0, 512), (512, D)):
                        for g in range(DG):
                            nc.tensor.matmul(
                                q2p[:, lo:hi],
                                qT[:, g:g + 1],
                                w2_sb[:, (DG + g) * D + lo:
                                      (DG + g) * D + hi],
                                start=(g == 0), stop=(g == DG - 1))
                    nc.vector.tensor_add(qb2r[:], q2p[:], b2row[:])

                # two pairs interleaved: pair a's matmuls fill the PE
                # bubbles left by pair b's DVE dependency chain
                pending_fea = []

                for pp in range(0, P_PER, 2):
                    duo = (pp, pp + 1)
                    st = {}
                    for p in duo:
                        A_sb, F_sb = staged[p]
                        # unpack 4 adjacency bits/byte -> fp8 {0,1}:
                        # plane k writes every 4th column, reconstructing
                        # the original column order
                        au = aupool.tile([128, NCH * N], F8, tag="au")
                        for k in range(4):
                            nc.vector.tensor_scalar(
                                au[:, k::4], A_sb[:], k, 1,
                                op0=mybir.AluOpType.logical_shift_right,
                                op1=mybir.AluOpType.bitwise_and)
                        s0c = s0_sb[:, p * NCH:(p + 1) * NCH]
                        wcol = spool.tile([128, NCH], F32, tag="wcol")
                        nc.vector.tensor_scalar_mul(wcol[:], s0c, c_init)
                        ct = spool.tile([128, NCH], F32, tag="ct")
                        st[p] = dict(A=au, F=F_sb, s0c=s0c, pcur=None,
                                     pref=None, wcol=wcol, ct=ct)

                    for t in range(STEP_NUM):
                        for p in duo:
                            s = st[p]
                            mov = s["pcur"] if t > 0 else s["s0c"]
                            ps = pp_psum.tile([128, NCH], F32, tag="pp")
                            s["ps"] = ps
                            for oc in range(NCH):
                                base = oc * 128
                                for ic in range(NCH):
                                    nc.tensor.matmul(
                                        ps[:, oc:oc + 1],
                                        s["A"][:, ic * N + base:
                                               ic * N + base + 128],
                                        mov[:, ic:ic + 1],
                                        start=(ic == 0),
                                        stop=(ic == NCH - 1),
                                    )
                        for p in duo:
                            s = st[p]
                            ps = s["ps"]
                            if t < STEP_NUM - 1:
                                # clamp to {0,1} so the fp8 cast is exact
                                # (e4m3 overflows above 448; counts can)
                                pnext = spool.tile([128, NCH], F8,
                                                   tag="pcur")
                                nc.vector.tensor_scalar_min(pnext[:],
                                                            ps[:], 1.0)
                                s["pcur"] = pnext
                            if t == 0:
                                pref = spool.tile([128, NCH], F32,
                                                  tag="pref")
                                nc.vector.tensor_add(pref[:], ps[:],
                                                     s["s0c"])
                                s["pref"] = pref
                            else:
                                nc.vector.tensor_add(s["pref"][:],
                                                     s["pref"][:], ps[:])
                            nc.vector.tensor_scalar_min(s["ct"][:],
                                                        s["pref"][:], 1.0)
                            nc.vector.scalar_tensor_tensor(
                                s["wcol"][:], s["ct"][:], coefs[t],
                                s["wcol"][:], op0=mult, op1=add)

                    for p in duo:
                        s = st[p]
                        dps = den_psum.tile([1, NCH], F32, tag="den")
                        nc.tensor.matmul(dps[:], ones128[:], s["ct"][:],
                                         start=True, stop=True)
                        den = spool.tile([1, 1], F32, tag="dens")
                        nc.vector.tensor_reduce(den[:], dps[:],
                                                axis=mybir.AxisListType.X,
                                                op=add)
                        # coverage count is an integer >= 1 unless the seed
                        # set is empty (w == 0 there, so any scale gives 0)
                        nc.vector.tensor_scalar_max(den[:], den[:], 0.5)
                        rec = spool.tile([1, 1], F32, tag="rec")
                        nc.vector.reciprocal(rec[:], den[:])
                        tmp = spool.tile([1, 1], F32, tag="tmp")
                        # fold mask and the ALPHA^4 rescale into one scalar
                        nc.vector.scalar_tensor_tensor(
                            tmp[:], rec[:], a4, mask_sb[:, p:p + 1],
                            op0=mult, op1=mult)
                        # broadcast the per-pair scalar to 128 partitions
                        # (activation scale must match the partition dim)
                        bcp = den_psum.tile([128, 1], F32, tag="den")
                        nc.tensor.matmul(bcp[:], onesrow[:], tmp[:],
                                         start=True, stop=True)
                        inv = spool.tile([128, 1], F32, tag="inv")
                        nc.scalar.copy(inv[:], bcp[:])
                        s["inv"] = inv
                        ubf = spool.tile([128, NCH], BF16, tag="ubf")
                        nc.vector.tensor_copy(ubf[:], s["wcol"][:])
                        s["ubf"] = ubf

                    # software pipeline: emit the PREVIOUS duo's fea
                    # here, so it fills the PE wait on this duo's DVE
                    # dependency chain (ubf/inv). F-stationary matmuls:
                    # output free size 1 -> near-zero PE time, and the
                    # column (transposed) layout falls out directly.
                    def emit_fea(pd, sd):
                        nfp = fea_psum.tile([128, DG], F32, tag="fea")
                        for g in range(DG):
                            for ic in range(NCH):
                                nc.tensor.matmul(
                                    nfp[:, g:g + 1],
                                    sd["F"][:, ic * D + g * 128:
                                            ic * D + g * 128 + 128],
                                    sd["ubf"][:, ic:ic + 1],
                                    start=(ic == 0),
                                    stop=(ic == NCH - 1),
                                )
                        nc.scalar.activation(nfT[:, pd::P_PER], nfp[:],
                                             relu, scale=sd["inv"][:])

                    for pd, sd in pending_fea:
                        emit_fea(pd, sd)
                    pending_fea = [(p, st[p]) for p in duo]

                    if pp == 6:
                        q_block()

                for pd, sd in pending_fea:
                    emit_fea(pd, sd)

            with (
                tc.tile_pool(name="mlpps", bufs=2, space="PSUM") as mps,
                tc.tile_pool(name="trps", bufs=1, space="PSUM") as tps,
            ):
                # nfT was filled per-pair inside the loop; all stages
                # use weight-stationary matmuls whose outputs are already
                # in transposed (column) layout. Biases fold in as one
                # extra bias-row x ones matmul per accumulation group so
                # each layer needs only ONE wide activation (fewer
                # serialized cross-engine hops on the critical tail).
                h1T = mpool.tile([128, DG * P_PER], BF16)
                hp1 = mps.tile([128, DG * P_PER], F32, tag="h")
                for go in range(DG):
                    cols = slice(go * P_PER, (go + 1) * P_PER)
                    for g in range(DG):
                        nc.tensor.matmul(
                            hp1[:, cols],
                            w1_sb[:, g * D + go * 128:g * D + go * 128 + 128],
                            nfT[:, g * P_PER:(g + 1) * P_PER],
                            start=(g == 0), stop=False)
                    nc.tensor.matmul(hp1[:, cols],
                                     b1row[0:1, go * 128:go * 128 + 128],
                                     ones16[:], start=False, stop=True)
                nc.scalar.activation(h1T[:], hp1[:], relu)

                h2T = mpool.tile([128, DG * P_PER], BF16)
                hp2 = mps.tile([128, DG * P_PER], F32, tag="h")
                for go in range(DG):
                    cols = slice(go * P_PER, (go + 1) * P_PER)
                    for g in range(DG):
                        nc.tensor.matmul(
                            hp2[:, cols],
                            w2_sb[:, g * D + go * 128:g * D + go * 128 + 128],
                            h1T[:, g * P_PER:(g + 1) * P_PER],
                            start=(g == 0), stop=False)
                    nc.tensor.matmul(hp2[:, cols],
                                     qb2r[0:1, go * 128:go * 128 + 128],
                                     ones16[:], start=False, stop=True)
                nc.scalar.activation(h2T[:], hp2[:], relu)

                yp = tps.tile([128, 1], F32, tag="tr")
                for g in range(DG):
                    nc.tensor.matmul(yp[0:P_PER, 0:1],
                                     h2T[:, g * P_PER:(g + 1) * P_PER],
                                     w3_sb[:, g:g + 1],
                                     start=(g == 0), stop=False)
                nc.tensor.matmul(yp[0:P_PER, 0:1], ones16[:], b3_sb[:],
                                 start=False, stop=True)
                ysb = mpool.tile([P_PER, 1], F32)
                nc.vector.tensor_copy(ysb[:], yp[0:P_PER, 0:1])
                nc.scalar.dma_start(y_out[:], ysb[:])

    nc.compile()
    return nc


def get_program():
    global _PROG
    if _PROG is None:
        _PROG = _build()
    return _PROG


def _pack4(bits):
    """Pack binary {0,1} float array [128, M] into 4-entries-per-byte u8:
    byte j = sum_k bits[:, 4j+k] << k (matches the DVE shift+mask unpack)."""
    b = bits.astype(np.uint8)
    return (b[:, 0::4] | (b[:, 1::4] << 1) | (b[:, 2::4] << 2)
            | (b[:, 3::4] << 3))


def _prep_core(core, query_fea, a_nei, vec_nei, fea_emb, nei_mask,
               W1, b1, W2, b2, W3, b3):
    b = (core * P_PER) // A
    a0 = (core * P_PER) % A
    a_loc = a_nei[b, a0:a0 + P_PER]
    f_loc = fea_emb[b, a0:a0 + P_PER]
    s_loc = vec_nei[b, a0:a0 + P_PER]
    return {
        "a_pre": _pack4(np.ascontiguousarray(
            a_loc.reshape(P_PER, NCH, 128, N).transpose(2, 0, 1, 3)
            .reshape(128, P_PER * NCH * N))),
        "f_pre": np.ascontiguousarray(
            f_loc.reshape(P_PER, NCH, 128, D).transpose(2, 0, 1, 3)
            .reshape(128, P_PER * NCH * D)).astype(BF16_NP),
        "s0_pre": np.ascontiguousarray(
            s_loc.reshape(P_PER, NCH, 128).transpose(2, 0, 1)
            .reshape(128, P_PER * NCH)).astype(F8_NP),
        "mask_pre": nei_mask[b, a0:a0 + P_PER, 0].reshape(1, P_PER)
        .astype(np.float32),
        "q_pre": query_fea[b].astype(BF16_NP),
        "w1_pre": np.ascontiguousarray(
            W1.reshape(DG, 128, D).transpose(1, 0, 2).reshape(128, DG * D))
        .astype(BF16_NP),
        "w2_pre": np.ascontiguousarray(
            W2.reshape(2 * DG, 128, D).transpose(1, 0, 2)
            .reshape(128, 2 * DG * D)).astype(BF16_NP),
        "w3_pre": np.ascontiguousarray(
            W3[:, 0].reshape(DG, 128).transpose(1, 0)).astype(BF16_NP),
        "b1_pre": b1.reshape(1, D).astype(BF16_NP),
        "b2_pre": b2.reshape(1, D).astype(np.float32),
        "b3_pre": b3.reshape(1, 1).astype(BF16_NP),
    }


_EXEC = None


def _make_exec():
    """Replicates bass2jax.run_bass_via_pjrt's multi-core path, but caches
    the jitted executable so repeated calls (and timing loops) skip
    recompilation."""
    global _EXEC
    if _EXEC is not None:
        return _EXEC
    import jax
    from jax.experimental.shard_map import shard_map
    from jax.sharding import Mesh, PartitionSpec

    from concourse import mybir as _mybir
    from concourse.bass2jax import (_bass_exec_p, install_neuronx_cc_hook,
                                    partition_id_tensor)

    nc = get_program()
    install_neuronx_cc_hook()
    partition_name = (nc.partition_id_tensor.name
                      if nc.partition_id_tensor else None)
    in_names, out_names, out_avals, zero_outs = [], [], [], []
    for alloc in nc.m.functions[0].allocations:
        if not isinstance(alloc, _mybir.MemoryLocationSet):
            continue
        name = alloc.memorylocations[0].name
        if alloc.kind == "ExternalInput":
            if name != partition_name:
                in_names.append(name)
        elif alloc.kind == "ExternalOutput":
            shape = tuple(alloc.tensor_shape)
            dtype = _mybir.dt.np(alloc.dtype)
            out_names.append(name)
            out_avals.append(jax.core.ShapedArray(shape, dtype))
            zero_outs.append(np.zeros(shape, dtype))
    n_params = len(in_names)
    all_in_names = list(in_names) + list(out_names)
    if partition_name is not None:
        all_in_names.append(partition_name)

    def _body(*args):
        operands = list(args)
        if partition_name is not None:
            operands.append(partition_id_tensor())
        outs = _bass_exec_p.bind(
            *operands,
            out_avals=tuple(out_avals),
            in_names=tuple(all_in_names),
            out_names=tuple(out_names),
            lowering_input_output_aliases=(),
            sim_require_finite=True,
            sim_require_nnan=True,
            nc=nc,
        )
        return tuple(outs)

    devices = jax.devices()[:NCORES]
    mesh = Mesh(np.asarray(devices), ("core",))
    n_outs = len(out_names)
    sharded = jax.jit(
        shard_map(_body, mesh=mesh,
                  in_specs=(PartitionSpec("core"),) * (n_params + n_outs),
                  out_specs=(PartitionSpec("core"),) * n_outs,
                  check_rep=False),
        keep_unused=True,
    )
    _EXEC = (sharded, in_names, out_names, out_avals, zero_outs, mesh)
    return _EXEC


def run_sharded(in_maps, reps=1):
    """Execute on 8 cores; returns (per-core results, [wall_ns per rep])."""
    import time as _time

    import jax

    sharded, in_names, out_names, out_avals, zero_outs, mesh = _make_exec()
    from jax.sharding import NamedSharding, PartitionSpec
    shard = NamedSharding(mesh, PartitionSpec("core"))
    concat_in = [
        jax.device_put(
            np.concatenate([np.asarray(in_maps[c][n])
                            for c in range(NCORES)], axis=0), shard)
        for n in in_names
    ]
    concat_zeros = [
        jax.device_put(
            np.zeros((NCORES * z.shape[0], *z.shape[1:]), z.dtype), shard)
        for z in zero_outs
    ]
    args = concat_in + concat_zeros
    jax.block_until_ready(args)
    out_arrs = None
    times = []
    for _ in range(max(1, reps)):
        t0 = _time.perf_counter()
        out_arrs = sharded(*args)
        jax.block_until_ready(out_arrs)
        times.append((_time.perf_counter() - t0) * 1e9)
    results = [
        {
            name: np.asarray(out_arrs[i]).reshape(
                NCORES, *out_avals[i].shape)[c]
            for i, name in enumerate(out_names)
        }
        for c in range(NCORES)
    ]
    return results, times


def kernel(query_fea, a_nei, vec_nei, fea_emb, nei_mask,
           W1, b1, W2, b2, W3, b3, trace=False, reps=1):
    global LAST_RESULT
    args = [np.asarray(x) for x in (query_fea, a_nei, vec_nei, fea_emb,
                                    nei_mask, W1, b1, W2, b2, W3, b3)]
    in_maps = [_prep_core(c, *args) for c in range(NCORES)]
    results, times = run_sharded(in_maps, reps=reps)
    LAST_RESULT = {"times_ns": times}
    ys = [results[c]["y"].reshape(P_PER) for c in range(NCORES)]
    return np.concatenate(ys).reshape(B, A, 1).astype(np.float32)


# revision 29
# speedup vs baseline: 1.1235x; 1.1235x over previous
"""Trainium2 Bass kernel for nn_DeepQNetIVCML (GNN message passing).

Reference computation per (b, a) pair:
  multi-hop coverage over a sparse binary adjacency (3 steps), weighted
  feature aggregation, mask + mean-normalize, then a small shared MLP.

Sharding: 128 (b, a) pairs split across 8 cores (16 pairs each; every
core sees exactly one b). MLP weights are replicated.

Key kernel ideas:
  - Propagation runs in "path count" space: p_{t+1} = A^T p_t with no
    thresholding between steps (support(prefix_sum) is exact), so
    cover_t = min(prefix_sum, 1) and the per-node weight is a telescoped
    linear combination of covers.
  - Adjacency and seed vectors are binary -> exact in fp8 e4m3.
    A-stationary matmuls keep the state in column layout.
  - fea = F^T w with F (fea_emb) streamed ONCE in bf16 (the 2e-2
    rel-err budget dwarfs bf16's ~4e-3), computed with the F tile as
    the matmul stationary operand so each matmul has output free size
    1 -> near-zero PE engine time and the result lands directly in
    transposed (column) layout; no transposes anywhere.
  - The per-node weights divided by ALPHA^4 are exact dyadic rationals
    (ALPHA = 0.8 -> {1.953125, 1.5625, 1.25, 1}), exact in bf16; ALPHA^4
    folds into the per-pair scalar.
  - mask/denominator/ALPHA^4 fold into one per-pair scalar, broadcast
    to 128 partitions with a 1-row matmul and applied as the activation
    scale at the relu.
  - MLP weights, query features and all MLP activations are bf16
    (halves their DMA bytes and 4x's the PE matmul rate vs f32).
  - DMA order: s0/mask ride the ACT ring first; the 16 pairs' A/F
    tiles stream on the SP ring with the MLP weight loads interleaved
    mid-stream so they neither delay the first pairs nor gate the tail.
"""

import os
import sys

for _p in ("/opt/trn_rl_repo", "/opt/pypackages"):
    if os.path.isdir(_p) and _p not in sys.path:
        sys.path.insert(0, _p)

import ml_dtypes
import numpy as np

import concourse.bacc as bacc
import concourse.mybir as mybir
from concourse.tile import TileContext

B, A, N, D, L = 4, 32, 512, 768, 128
ALPHA = 0.8
STEP_NUM = 3
NCORES = 8
P_PER = (B * A) // NCORES  # pairs per core
NCH = N // 128             # node chunks
DG = D // 128              # feature chunks

BF16 = mybir.dt.bfloat16
F8 = mybir.dt.float8e4
U8 = mybir.dt.uint8
F32 = mybir.dt.float32
BF16_NP = ml_dtypes.bfloat16
F8_NP = ml_dtypes.float8_e4m3

_PROG = None
LAST_RESULT = None


def _build():
    nc = bacc.Bacc("TRN2", target_bir_lowering=False, debug=False,
                   num_devices=NCORES)

    # adjacency bit-packed 4 entries/byte: 4x less DMA traffic, unpacked
    # on-chip by DVE shift+mask ops (DVE has headroom; DMA is the
    # bottleneck)
    a_pre = nc.dram_tensor("a_pre", [128, P_PER * NCH * N // 4], U8,
                           kind="ExternalInput")
    f_pre = nc.dram_tensor("f_pre", [128, P_PER * NCH * D], BF16,
                           kind="ExternalInput")
    s0_pre = nc.dram_tensor("s0_pre", [128, P_PER * NCH], F8,
                            kind="ExternalInput")
    mask_pre = nc.dram_tensor("mask_pre", [1, P_PER], F32,
                              kind="ExternalInput")
    q_pre = nc.dram_tensor("q_pre", [L, D], BF16, kind="ExternalInput")
    w1_pre = nc.dram_tensor("w1_pre", [128, DG * D], BF16,
                            kind="ExternalInput")
    w2_pre = nc.dram_tensor("w2_pre", [128, 2 * DG * D], BF16,
                            kind="ExternalInput")
    w3_pre = nc.dram_tensor("w3_pre", [128, DG], BF16, kind="ExternalInput")
    b1_pre = nc.dram_tensor("b1_pre", [1, D], BF16, kind="ExternalInput")
    b2_pre = nc.dram_tensor("b2_pre", [1, D], F32, kind="ExternalInput")
    b3_pre = nc.dram_tensor("b3_pre", [1, 1], BF16, kind="ExternalInput")
    y_out = nc.dram_tensor("y", [P_PER, 1], F32, kind="ExternalOutput")

    mult = mybir.AluOpType.mult
    add = mybir.AluOpType.add
    relu = mybir.ActivationFunctionType.Relu

    # per-cover weights scaled by ALPHA^-4: exact dyadic rationals
    c_init = 1.0 / ALPHA**3 - 1.0 / ALPHA**2       # 0.390625
    coefs = [1.0 / ALPHA**2 - 1.0 / ALPHA,         # 0.3125
             1.0 / ALPHA - 1.0,                    # 0.25
             1.0]
    a4 = float(np.float32(ALPHA) ** 4)

    with TileContext(nc) as tc:
        with (
            tc.tile_pool(name="const", bufs=1) as cpool,
            tc.tile_pool(name="weights", bufs=1) as wpool,
            tc.tile_pool(name="aubuf", bufs=6) as aupool,
            tc.tile_pool(name="fbuf", bufs=16) as fpool,
            tc.tile_pool(name="small", bufs=16) as spool,
            tc.tile_pool(name="mlp", bufs=1) as mpool,
        ):
            ones16 = cpool.tile([1, P_PER], BF16)
            nc.vector.memset(ones16[:], 1.0)
            onesL = cpool.tile([128, 1], BF16)
            nc.vector.memset(onesL[:], 1.0 / L)
            ones128 = cpool.tile([128, 1], F32)
            nc.vector.memset(ones128[:], 1.0)
            onesrow = cpool.tile([1, 128], F32)
            nc.vector.memset(onesrow[:], 1.0)

            # the SP HWDGE ring is FIFO. All 16 adjacency tiles go first
            # (11.6us) so every pair's propagation/coverage chain
            # completes mid-stream; then the feature tiles stream with
            # the MLP weight loads interleaved so they neither delay the
            # first features nor arrive after the pair stream ends. The
            # tail after the last F tile is then just fea + MLP.
            # ALL packed adjacency rides ONE big DMA: 16 per-pair copies
            # would each pay the ~650ns HWDGE slot (transfers are only
            # 182ns), while one 2913ns transfer also lets the HWDGE run
            # ahead so the small s0/mask/q copies slot in behind it for
            # free.
            NP4 = NCH * N // 4
            a_all = cpool.tile([128, P_PER * NP4], U8)
            nc.sync.dma_start(a_all[:], a_pre[:])

            # tiny state/mask loads ride right behind the A block (the
            # propagation that needs them has tens of us of slack)
            s0_sb = cpool.tile([128, P_PER * NCH], F8)
            nc.sync.dma_start(s0_sb[:], s0_pre[:])
            mask_sb = cpool.tile([1, P_PER], F32)
            nc.sync.dma_start(mask_sb[:], mask_pre[:])

            staged = {}

            def f_dma(p):
                F_sb = fpool.tile([128, NCH * D], BF16, tag="F")
                nc.sync.dma_start(F_sb[:],
                                  f_pre[:, p * NCH * D:(p + 1) * NCH * D])
                staged[p] = F_sb

            # q goes first among the F-stream inserts: the q_block's qT
            # copies sit on the ACT queue ahead of later nfT
            # activations, so a late q load would stall the activation
            # queue and with it the fea psum rotation.
            q_sb = cpool.tile([L, D], BF16)
            nc.sync.dma_start(q_sb[:], q_pre[:])
            f_dma(0)
            f_dma(1)
            w3_sb = wpool.tile([128, DG], BF16)
            nc.sync.dma_start(w3_sb[:], w3_pre[:])
            b1row = cpool.tile([1, D], BF16)
            nc.sync.dma_start(b1row[:], b1_pre[:])
            b2row = cpool.tile([1, D], F32)
            nc.sync.dma_start(b2row[:], b2_pre[:])
            b3_sb = cpool.tile([1, 1], BF16)
            nc.sync.dma_start(b3_sb[:], b3_pre[:])
            for p in range(2, 8):
                f_dma(p)
            w2_sb = wpool.tile([128, 2 * DG * D], BF16)
            nc.sync.dma_start(w2_sb[:], w2_pre[:])
            f_dma(8)
            f_dma(9)
            w1_sb = wpool.tile([128, DG * D], BF16)
            nc.sync.dma_start(w1_sb[:], w1_pre[:])
            for p in range(10, P_PER):
                f_dma(p)

            nfT = mpool.tile([128, DG * P_PER], BF16)

            with (
                tc.tile_pool(name="ppps", bufs=2, space="PSUM") as pp_psum,
                tc.tile_pool(name="feaps", bufs=2, space="PSUM") as fea_psum,
                tc.tile_pool(name="denps", bufs=1, space="PSUM") as den_psum,
                tc.tile_pool(name="qps", bufs=1, space="PSUM") as qpsum,
            ):
                # combined h2 bias row: W2q^T qbar + b2, as a [1, D] row
                # so the MLP can fold it in as one bias matmul per group
                qb2r = mpool.tile([1, D], BF16)

                def q_block():
                    # q-side of the MLP: placed mid-loop in PE program
                    # order so its weight-DMA waits never head-block the
                    # PE instruction FIFO. Weight-stationary orientation
                    # produces the transposed column layout directly.
                    qT = mpool.tile([128, DG], BF16)
                    qtp = qpsum.tile([128, 1], F32, tag="qt")
                    for g in range(DG):
                        nc.tensor.matmul(qtp[:],
                                         q_sb[:, g * 128:(g + 1) * 128],
                                         onesL[:], start=True, stop=True)
                        nc.scalar.copy(qT[:, g:g + 1], qtp[:])
                    q2p = qpsum.tile([1, D], F32, tag="q2")
                    for lo, hi in ((0, 512), (512, D)):
                        for g in range(DG):
                            nc.tensor.matmul(
                                q2p[:, lo:hi],
                                qT[:, g:g + 1],
                                w2_sb[:, (DG + g) * D + lo:
                                      (DG + g) * D + hi],
                                start=(g == 0), stop=(g == DG - 1))
                    nc.vector.tensor_add(qb2r[:], q2p[:], b2row[:])

                # two pairs interleaved: pair a's matmuls fill the PE
                # bubbles left by pair b's DVE dependency chain
                pending_fea = []

                for pp in range(0, P_PER, 2):
                    duo = (pp, pp + 1)
                    st = {}
                    for p in duo:
                        F_sb = staged[p]
                        # unpack 4 adjacency bits/byte -> fp8 {0,1}:
                        # plane k writes every 4th column, reconstructing
                        # the original column order
                        au = aupool.tile([128, NCH * N], F8, tag="au")
                        for k in range(4):
                            nc.vector.tensor_scalar(
                                au[:, k::4],
                                a_all[:, p * NP4:(p + 1) * NP4], k, 1,
                                op0=mybir.AluOpType.logical_shift_right,
                                op1=mybir.AluOpType.bitwise_and)
                        s0c = s0_sb[:, p * NCH:(p + 1) * NCH]
                        wcol = spool.tile([128, NCH], F32, tag="wcol")
                        nc.vector.tensor_scalar_mul(wcol[:], s0c, c_init)
                        ct = spool.tile([128, NCH], F32, tag="ct")
                        st[p] = dict(A=au, F=F_sb, s0c=s0c, pcur=None,
                                     pref=None, wcol=wcol, ct=ct)

                    for t in range(STEP_NUM):
                        for p in duo:
                            s = st[p]
                            mov = s["pcur"] if t > 0 else s["s0c"]
                            ps = pp_psum.tile([128, NCH], F32, tag="pp")
                            s["ps"] = ps
                            for oc in range(NCH):
                                base = oc * 128
                                for ic in range(NCH):
                                    nc.tensor.matmul(
                                        ps[:, oc:oc + 1],
                                        s["A"][:, ic * N + base:
                                               ic * N + base + 128],
                                        mov[:, ic:ic + 1],
                                        start=(ic == 0),
                                        stop=(ic == NCH - 1),
                                    )
                        for p in duo:
                            s = st[p]
                            ps = s["ps"]
                            if t < STEP_NUM - 1:
                                # clamp to {0,1} so the fp8 cast is exact
                                # (e4m3 overflows above 448; counts can)
                                pnext = spool.tile([128, NCH], F8,
                                                   tag="pcur")
                                nc.vector.tensor_scalar_min(pnext[:],
                                                            ps[:], 1.0)
                                s["pcur"] = pnext
                            if t == 0:
                                pref = spool.tile([128, NCH], F32,
                                                  tag="pref")
                                nc.vector.tensor_add(pref[:], ps[:],
                                                     s["s0c"])
                                s["pref"] = pref
                            else:
                                nc.vector.tensor_add(s["pref"][:],
                                                     s["pref"][:], ps[:])
                            nc.vector.tensor_scalar_min(s["ct"][:],
                                                        s["pref"][:], 1.0)
                            nc.vector.scalar_tensor_tensor(
                                s["wcol"][:], s["ct"][:], coefs[t],
                                s["wcol"][:], op0=mult, op1=add)

                    for p in duo:
                        s = st[p]
                        dps = den_psum.tile([1, NCH], F32, tag="den")
                        nc.tensor.matmul(dps[:], ones128[:], s["ct"][:],
                                         start=True, stop=True)
                        den = spool.tile([1, 1], F32, tag="dens")
                        nc.vector.tensor_reduce(den[:], dps[:],
                                                axis=mybir.AxisListType.X,
                                                op=add)
                        # coverage count is an integer >= 1 unless the seed
                        # set is empty (w == 0 there, so any scale gives 0)
                        nc.vector.tensor_scalar_max(den[:], den[:], 0.5)
                        rec = spool.tile([1, 1], F32, tag="rec")
                        nc.vector.reciprocal(rec[:], den[:])
                        tmp = spool.tile([1, 1], F32, tag="tmp")
                        # fold mask and the ALPHA^4 rescale into one scalar
                        nc.vector.scalar_tensor_tensor(
                            tmp[:], rec[:], a4, mask_sb[:, p:p + 1],
                            op0=mult, op1=mult)
                        # broadcast the per-pair scalar to 128 partitions
                        # (activation scale must match the partition dim)
                        bcp = den_psum.tile([128, 1], F32, tag="den")
                        nc.tensor.matmul(bcp[:], onesrow[:], tmp[:],
                                         start=True, stop=True)
                        inv = spool.tile([128, 1], F32, tag="inv")
                        nc.scalar.copy(inv[:], bcp[:])
                        s["inv"] = inv
                        ubf = spool.tile([128, NCH], BF16, tag="ubf")
                        nc.vector.tensor_copy(ubf[:], s["wcol"][:])
                        s["ubf"] = ubf

                    # software pipeline: emit the PREVIOUS duo's fea
                    # here, so it fills the PE wait on this duo's DVE
                    # dependency chain (ubf/inv). F-stationary matmuls:
                    # output free size 1 -> near-zero PE time, and the
                    # column (transposed) layout falls out directly.
                    def emit_fea(pd, sd):
                        nfp = fea_psum.tile([128, DG], F32, tag="fea")
                        for g in range(DG):
                            for ic in range(NCH):
                                nc.tensor.matmul(
                                    nfp[:, g:g + 1],
                                    sd["F"][:, ic * D + g * 128:
                                            ic * D + g * 128 + 128],
                                    sd["ubf"][:, ic:ic + 1],
                                    start=(ic == 0),
                                    stop=(ic == NCH - 1),
                                )
                        nc.scalar.activation(nfT[:, pd::P_PER], nfp[:],
                                             relu, scale=sd["inv"][:])

                    for pd, sd in pending_fea:
                        emit_fea(pd, sd)
                    pending_fea = [(p, st[p]) for p in duo]

                    if pp == 6:
                        q_block()

                for pd, sd in pending_fea:
                    emit_fea(pd, sd)

            with (
                tc.tile_pool(name="mlpps", bufs=2, space="PSUM") as mps,
                tc.tile_pool(name="trps", bufs=1, space="PSUM") as tps,
            ):
                # nfT was filled per-pair inside the loop; all stages
                # use weight-stationary matmuls whose outputs are already
                # in transposed (column) layout. Biases fold in as one
                # extra bias-row x ones matmul per accumulation group so
                # each layer needs only ONE wide activation (fewer
                # serialized cross-engine hops on the critical tail).
                h1T = mpool.tile([128, DG * P_PER], BF16)
                hp1 = mps.tile([128, DG * P_PER], F32, tag="h")
                for go in range(DG):
                    cols = slice(go * P_PER, (go + 1) * P_PER)
                    for g in range(DG):
                        nc.tensor.matmul(
                            hp1[:, cols],
                            w1_sb[:, g * D + go * 128:g * D + go * 128 + 128],
                            nfT[:, g * P_PER:(g + 1) * P_PER],
                            start=(g == 0), stop=False)
                    nc.tensor.matmul(hp1[:, cols],
                                     b1row[0:1, go * 128:go * 128 + 128],
                                     ones16[:], start=False, stop=True)
                nc.scalar.activation(h1T[:], hp1[:], relu)

                h2T = mpool.tile([128, DG * P_PER], BF16)
                hp2 = mps.tile([128, DG * P_PER], F32, tag="h")
                for go in range(DG):
                    cols = slice(go * P_PER, (go + 1) * P_PER)
                    for g in range(DG):
                        nc.tensor.matmul(
                            hp2[:, cols],
                            w2_sb[:, g * D + go * 128:g * D + go * 128 + 128],
                            h1T[:, g * P_PER:(g + 1) * P_PER],
                            start=(g == 0), stop=False)
                    nc.tensor.matmul(hp2[:, cols],
                                     qb2r[0:1, go * 128:go * 128 + 128],
                                     ones16[:], start=False, stop=True)
                nc.scalar.activation(h2T[:], hp2[:], relu)

                yp = tps.tile([128, 1], F32, tag="tr")
                for g in range(DG):
                    nc.tensor.matmul(yp[0:P_PER, 0:1],
                                     h2T[:, g * P_PER:(g + 1) * P_PER],
                                     w3_sb[:, g:g + 1],
                                     start=(g == 0), stop=False)
                nc.tensor.matmul(yp[0:P_PER, 0:1], ones16[:], b3_sb[:],
                                 start=False, stop=True)
                ysb = mpool.tile([P_PER, 1], F32)
                nc.vector.tensor_copy(ysb[:], yp[0:P_PER, 0:1])
                nc.scalar.dma_start(y_out[:], ysb[:])

    nc.compile()
    return nc


def get_program():
    global _PROG
    if _PROG is None:
        _PROG = _build()
    return _PROG


def _pack4(bits):
    """Pack binary {0,1} float array [128, M] into 4-entries-per-byte u8:
    byte j = sum_k bits[:, 4j+k] << k (matches the DVE shift+mask unpack)."""
    b = bits.astype(np.uint8)
    return (b[:, 0::4] | (b[:, 1::4] << 1) | (b[:, 2::4] << 2)
            | (b[:, 3::4] << 3))


def _prep_core(core, query_fea, a_nei, vec_nei, fea_emb, nei_mask,
               W1, b1, W2, b2, W3, b3):
    b = (core * P_PER) // A
    a0 = (core * P_PER) % A
    a_loc = a_nei[b, a0:a0 + P_PER]
    f_loc = fea_emb[b, a0:a0 + P_PER]
    s_loc = vec_nei[b, a0:a0 + P_PER]
    return {
        "a_pre": _pack4(np.ascontiguousarray(
            a_loc.reshape(P_PER, NCH, 128, N).transpose(2, 0, 1, 3)
            .reshape(128, P_PER * NCH * N))),
        "f_pre": np.ascontiguousarray(
            f_loc.reshape(P_PER, NCH, 128, D).transpose(2, 0, 1, 3)
            .reshape(128, P_PER * NCH * D)).astype(BF16_NP),
        "s0_pre": np.ascontiguousarray(
            s_loc.reshape(P_PER, NCH, 128).transpose(2, 0, 1)
            .reshape(128, P_PER * NCH)).astype(F8_NP),
        "mask_pre": nei_mask[b, a0:a0 + P_PER, 0].reshape(1, P_PER)
        .astype(np.float32),
        "q_pre": query_fea[b].astype(BF16_NP),
        "w1_pre": np.ascontiguousarray(
            W1.reshape(DG, 128, D).transpose(1, 0, 2).reshape(128, DG * D))
        .astype(BF16_NP),
        "w2_pre": np.ascontiguousarray(
            W2.reshape(2 * DG, 128, D).transpose(1, 0, 2)
            .reshape(128, 2 * DG * D)).astype(BF16_NP),
        "w3_pre": np.ascontiguousarray(
            W3[:, 0].reshape(DG, 128).transpose(1, 0)).astype(BF16_NP),
        "b1_pre": b1.reshape(1, D).astype(BF16_NP),
        "b2_pre": b2.reshape(1, D).astype(np.float32),
        "b3_pre": b3.reshape(1, 1).astype(BF16_NP),
    }


_EXEC = None


def _make_exec():
    """Replicates bass2jax.run_bass_via_pjrt's multi-core path, but caches
    the jitted executable so repeated calls (and timing loops) skip
    recompilation."""
    global _EXEC
    if _EXEC is not None:
        return _EXEC
    import jax
    from jax.experimental.shard_map import shard_map
    from jax.sharding import Mesh, PartitionSpec

    from concourse import mybir as _mybir
    from concourse.bass2jax import (_bass_exec_p, install_neuronx_cc_hook,
                                    partition_id_tensor)

    nc = get_program()
    install_neuronx_cc_hook()
    partition_name = (nc.partition_id_tensor.name
                      if nc.partition_id_tensor else None)
    in_names, out_names, out_avals, zero_outs = [], [], [], []
    for alloc in nc.m.functions[0].allocations:
        if not isinstance(alloc, _mybir.MemoryLocationSet):
            continue
        name = alloc.memorylocations[0].name
        if alloc.kind == "ExternalInput":
            if name != partition_name:
                in_names.append(name)
        elif alloc.kind == "ExternalOutput":
            shape = tuple(alloc.tensor_shape)
            dtype = _mybir.dt.np(alloc.dtype)
            out_names.append(name)
            out_avals.append(jax.core.ShapedArray(shape, dtype))
            zero_outs.append(np.zeros(shape, dtype))
    n_params = len(in_names)
    all_in_names = list(in_names) + list(out_names)
    if partition_name is not None:
        all_in_names.append(partition_name)

    def _body(*args):
        operands = list(args)
        if partition_name is not None:
            operands.append(partition_id_tensor())
        outs = _bass_exec_p.bind(
            *operands,
            out_avals=tuple(out_avals),
            in_names=tuple(all_in_names),
            out_names=tuple(out_names),
            lowering_input_output_aliases=(),
            sim_require_finite=True,
            sim_require_nnan=True,
            nc=nc,
        )
        return tuple(outs)

    devices = jax.devices()[:NCORES]
    mesh = Mesh(np.asarray(devices), ("core",))
    n_outs = len(out_names)
    sharded = jax.jit(
        shard_map(_body, mesh=mesh,
                  in_specs=(PartitionSpec("core"),) * (n_params + n_outs),
                  out_specs=(PartitionSpec("core"),) * n_outs,
                  check_rep=False),
        keep_unused=True,
    )
    _EXEC = (sharded, in_names, out_names, out_avals, zero_outs, mesh)
    return _EXEC


def run_sharded(in_maps, reps=1):
    """Execute on 8 cores; returns (per-core results, [wall_ns per rep])."""
    import time as _time

    import jax

    sharded, in_names, out_names, out_avals, zero_outs, mesh = _make_exec()
    from jax.sharding import NamedSharding, PartitionSpec
    shard = NamedSharding(mesh, PartitionSpec("core"))
    concat_in = [
        jax.device_put(
            np.concatenate([np.asarray(in_maps[c][n])
                            for c in range(NCORES)], axis=0), shard)
        for n in in_names
    ]
    concat_zeros = [
        jax.device_put(
            np.zeros((NCORES * z.shape[0], *z.shape[1:]), z.dtype), shard)
        for z in zero_outs
    ]
    args = concat_in + concat_zeros
    jax.block_until_ready(args)
    out_arrs = None
    times = []
    for _ in range(max(1, reps)):
        t0 = _time.perf_counter()
        out_arrs = sharded(*args)
        jax.block_until_ready(out_arrs)
        times.append((_time.perf_counter() - t0) * 1e9)
    results = [
        {
            name: np.asarray(out_arrs[i]).reshape(
                NCORES, *out_avals[i].shape)[c]
            for i, name in enumerate(out_names)
        }
        for c in range(NCORES)
    ]
    return results, times


def kernel(query_fea, a_nei, vec_nei, fea_emb, nei_mask,
           W1, b1, W2, b2, W3, b3, trace=False, reps=1):
    global LAST_RESULT
    args = [np.asarray(x) for x in (query_fea, a_nei, vec_nei, fea_emb,
                                    nei_mask, W1, b1, W2, b2, W3, b3)]
    in_maps = [_prep_core(c, *args) for c in range(NCORES)]
    results, times = run_sharded(in_maps, reps=reps)
    LAST_RESULT = {"times_ns": times}
    ys = [results[c]["y"].reshape(P_PER) for c in range(NCORES)]
    return np.concatenate(ys).reshape(B, A, 1).astype(np.float32)


# revision 31
# speedup vs baseline: 1.1526x; 1.0259x over previous
"""Trainium2 Bass kernel for nn_DeepQNetIVCML (GNN message passing).

Reference computation per (b, a) pair:
  multi-hop coverage over a sparse binary adjacency (3 steps), weighted
  feature aggregation, mask + mean-normalize, then a small shared MLP.

Sharding: 128 (b, a) pairs split across 8 cores (16 pairs each; every
core sees exactly one b). MLP weights are replicated.

Key kernel ideas:
  - Propagation runs in "path count" space: p_{t+1} = A^T p_t with no
    thresholding between steps (support(prefix_sum) is exact), so
    cover_t = min(prefix_sum, 1) and the per-node weight is a telescoped
    linear combination of covers.
  - Adjacency and seed vectors are binary -> exact in fp8 e4m3.
    A-stationary matmuls keep the state in column layout.
  - fea = F^T w with F (fea_emb) streamed ONCE in bf16 (the 2e-2
    rel-err budget dwarfs bf16's ~4e-3), computed with the F tile as
    the matmul stationary operand so each matmul has output free size
    1 -> near-zero PE engine time and the result lands directly in
    transposed (column) layout; no transposes anywhere.
  - The per-node weights divided by ALPHA^4 are exact dyadic rationals
    (ALPHA = 0.8 -> {1.953125, 1.5625, 1.25, 1}), exact in bf16; ALPHA^4
    folds into the per-pair scalar.
  - mask/denominator/ALPHA^4 fold into one per-pair scalar, broadcast
    to 128 partitions with a 1-row matmul and applied as the activation
    scale at the relu.
  - MLP weights, query features and all MLP activations are bf16
    (halves their DMA bytes and 4x's the PE matmul rate vs f32).
  - DMA order: s0/mask ride the ACT ring first; the 16 pairs' A/F
    tiles stream on the SP ring with the MLP weight loads interleaved
    mid-stream so they neither delay the first pairs nor gate the tail.
"""

import os
import sys

for _p in ("/opt/trn_rl_repo", "/opt/pypackages"):
    if os.path.isdir(_p) and _p not in sys.path:
        sys.path.insert(0, _p)

import ml_dtypes
import numpy as np

import concourse.bacc as bacc
import concourse.mybir as mybir
from concourse.tile import TileContext

B, A, N, D, L = 4, 32, 512, 768, 128
ALPHA = 0.8
STEP_NUM = 3
NCORES = 8
P_PER = (B * A) // NCORES  # pairs per core
NCH = N // 128             # node chunks
DG = D // 128              # feature chunks

BF16 = mybir.dt.bfloat16
F8 = mybir.dt.float8e4
U8 = mybir.dt.uint8
F32 = mybir.dt.float32
BF16_NP = ml_dtypes.bfloat16
F8_NP = ml_dtypes.float8_e4m3

_PROG = None
LAST_RESULT = None


def _build():
    nc = bacc.Bacc("TRN2", target_bir_lowering=False, debug=False,
                   num_devices=NCORES)

    # adjacency bit-packed 4 entries/byte: 4x less DMA traffic, unpacked
    # on-chip by DVE shift+mask ops (DVE has headroom; DMA is the
    # bottleneck)
    a_pre = nc.dram_tensor("a_pre", [128, P_PER * NCH * N // 4], U8,
                           kind="ExternalInput")
    f_pre = nc.dram_tensor("f_pre", [128, P_PER * NCH * D], BF16,
                           kind="ExternalInput")
    s0_pre = nc.dram_tensor("s0_pre", [128, P_PER * NCH], F8,
                            kind="ExternalInput")
    mask_pre = nc.dram_tensor("mask_pre", [1, P_PER], F32,
                              kind="ExternalInput")
    q_pre = nc.dram_tensor("q_pre", [L, D], BF16, kind="ExternalInput")
    w1_pre = nc.dram_tensor("w1_pre", [128, DG * D], BF16,
                            kind="ExternalInput")
    w2_pre = nc.dram_tensor("w2_pre", [128, 2 * DG * D], BF16,
                            kind="ExternalInput")
    w3_pre = nc.dram_tensor("w3_pre", [128, DG], BF16, kind="ExternalInput")
    b1_pre = nc.dram_tensor("b1_pre", [1, D], BF16, kind="ExternalInput")
    b2_pre = nc.dram_tensor("b2_pre", [1, D], F32, kind="ExternalInput")
    b3_pre = nc.dram_tensor("b3_pre", [1, 1], BF16, kind="ExternalInput")
    y_out = nc.dram_tensor("y", [P_PER, 1], F32, kind="ExternalOutput")

    mult = mybir.AluOpType.mult
    add = mybir.AluOpType.add
    relu = mybir.ActivationFunctionType.Relu

    # per-cover weights scaled by ALPHA^-4: exact dyadic rationals
    c_init = 1.0 / ALPHA**3 - 1.0 / ALPHA**2       # 0.390625
    coefs = [1.0 / ALPHA**2 - 1.0 / ALPHA,         # 0.3125
             1.0 / ALPHA - 1.0,                    # 0.25
             1.0]
    a4 = float(np.float32(ALPHA) ** 4)

    with TileContext(nc) as tc:
        with (
            tc.tile_pool(name="const", bufs=1) as cpool,
            tc.tile_pool(name="weights", bufs=1) as wpool,
            tc.tile_pool(name="aubuf", bufs=6) as aupool,
            tc.tile_pool(name="fbuf", bufs=16) as fpool,
            tc.tile_pool(name="small", bufs=16) as spool,
            tc.tile_pool(name="mlp", bufs=1) as mpool,
        ):
            ones16 = cpool.tile([1, P_PER], BF16)
            nc.vector.memset(ones16[:], 1.0)
            onesL = cpool.tile([128, 1], BF16)
            nc.vector.memset(onesL[:], 1.0 / L)
            ones128 = cpool.tile([128, 1], F8)
            nc.vector.memset(ones128[:], 1.0)
            onesrow = cpool.tile([1, 128], F32)
            nc.vector.memset(onesrow[:], 1.0)

            # the SP HWDGE ring is FIFO. All 16 adjacency tiles go first
            # (11.6us) so every pair's propagation/coverage chain
            # completes mid-stream; then the feature tiles stream with
            # the MLP weight loads interleaved so they neither delay the
            # first features nor arrive after the pair stream ends. The
            # tail after the last F tile is then just fea + MLP.
            # ALL packed adjacency rides ONE big DMA: 16 per-pair copies
            # would each pay the ~650ns HWDGE slot (transfers are only
            # 182ns), while one 2913ns transfer also lets the HWDGE run
            # ahead so the small s0/mask/q copies slot in behind it for
            # free.
            NP4 = NCH * N // 4
            a_all = cpool.tile([128, P_PER * NP4], U8)
            nc.sync.dma_start(a_all[:], a_pre[:])

            # tiny state/mask loads ride right behind the A block (the
            # propagation that needs them has tens of us of slack)
            s0_sb = cpool.tile([128, P_PER * NCH], F8)
            nc.sync.dma_start(s0_sb[:], s0_pre[:])
            mask_sb = cpool.tile([1, P_PER], F32)
            nc.sync.dma_start(mask_sb[:], mask_pre[:])

            staged = {}

            def f_dma(p):
                F_sb = fpool.tile([128, NCH * D], BF16, tag="F")
                nc.sync.dma_start(F_sb[:],
                                  f_pre[:, p * NCH * D:(p + 1) * NCH * D])
                staged[p] = F_sb

            # q goes first among the F-stream inserts: the q_block's qT
            # copies sit on the ACT queue ahead of later nfT
            # activations, so a late q load would stall the activation
            # queue and with it the fea psum rotation.
            q_sb = cpool.tile([L, D], BF16)
            nc.sync.dma_start(q_sb[:], q_pre[:])
            f_dma(0)
            f_dma(1)
            w3_sb = wpool.tile([128, DG], BF16)
            nc.sync.dma_start(w3_sb[:], w3_pre[:])
            b1row = cpool.tile([1, D], BF16)
            nc.sync.dma_start(b1row[:], b1_pre[:])
            b2row = cpool.tile([1, D], F32)
            nc.sync.dma_start(b2row[:], b2_pre[:])
            b3_sb = cpool.tile([1, 1], BF16)
            nc.sync.dma_start(b3_sb[:], b3_pre[:])
            for p in range(2, 8):
                f_dma(p)
            w2_sb = wpool.tile([128, 2 * DG * D], BF16)
            nc.sync.dma_start(w2_sb[:], w2_pre[:])
            f_dma(8)
            f_dma(9)
            w1_sb = wpool.tile([128, DG * D], BF16)
            nc.sync.dma_start(w1_sb[:], w1_pre[:])
            for p in range(10, P_PER):
                f_dma(p)

            nfT = mpool.tile([128, DG * P_PER], BF16)

            with (
                tc.tile_pool(name="ppps", bufs=2, space="PSUM") as pp_psum,
                tc.tile_pool(name="feaps", bufs=2, space="PSUM") as fea_psum,
                tc.tile_pool(name="denps", bufs=1, space="PSUM") as den_psum,
                tc.tile_pool(name="qps", bufs=1, space="PSUM") as qpsum,
            ):
                # combined h2 bias row: W2q^T qbar + b2, as a [1, D] row
                # so the MLP can fold it in as one bias matmul per group
                qb2r = mpool.tile([1, D], BF16)

                def q_block():
                    # q-side of the MLP: placed mid-loop in PE program
                    # order so its weight-DMA waits never head-block the
                    # PE instruction FIFO. Weight-stationary orientation
                    # produces the transposed column layout directly.
                    qT = mpool.tile([128, DG], BF16)
                    qtp = qpsum.tile([128, 1], F32, tag="qt")
                    for g in range(DG):
                        nc.tensor.matmul(qtp[:],
                                         q_sb[:, g * 128:(g + 1) * 128],
                                         onesL[:], start=True, stop=True)
                        nc.scalar.copy(qT[:, g:g + 1], qtp[:])
                    q2p = qpsum.tile([1, D], F32, tag="q2")
                    for lo, hi in ((0, 512), (512, D)):
                        for g in range(DG):
                            nc.tensor.matmul(
                                q2p[:, lo:hi],
                                qT[:, g:g + 1],
                                w2_sb[:, (DG + g) * D + lo:
                                      (DG + g) * D + hi],
                                start=(g == 0), stop=(g == DG - 1))
                    nc.vector.tensor_add(qb2r[:], q2p[:], b2row[:])

                # two pairs interleaved: pair a's matmuls fill the PE
                # bubbles left by pair b's DVE dependency chain
                pending_fea = []

                for pp in range(0, P_PER, 2):
                    duo = (pp, pp + 1)
                    st = {}
                    for p in duo:
                        F_sb = staged[p]
                        # unpack 4 adjacency bits/byte -> fp8 {0,1}:
                        # plane k writes every 4th column, reconstructing
                        # the original column order
                        au = aupool.tile([128, NCH * N], F8, tag="au")
                        for k in range(4):
                            nc.vector.tensor_scalar(
                                au[:, k::4],
                                a_all[:, p * NP4:(p + 1) * NP4], k, 1,
                                op0=mybir.AluOpType.logical_shift_right,
                                op1=mybir.AluOpType.bitwise_and)
                        s0c = s0_sb[:, p * NCH:(p + 1) * NCH]
                        wcol = spool.tile([128, NCH], F32, tag="wcol")
                        nc.vector.tensor_scalar_mul(wcol[:], s0c, c_init)
                        st[p] = dict(A=au, F=F_sb, s0c=s0c, ct=None,
                                     pref=None, wcol=wcol)

                    # propagate the cumulative cover itself (exact:
                    # support(pref + A^T cover) is still the <=t+1-hop
                    # set), so one fp8 min() per step yields both the
                    # next moving operand and the cover for the weights
                    for t in range(STEP_NUM):
                        for p in duo:
                            s = st[p]
                            mov = s["ct"] if t > 0 else s["s0c"]
                            ps = pp_psum.tile([128, NCH], F32, tag="pp")
                            s["ps"] = ps
                            for oc in range(NCH):
                                base = oc * 128
                                for ic in range(NCH):
                                    nc.tensor.matmul(
                                        ps[:, oc:oc + 1],
                                        s["A"][:, ic * N + base:
                                               ic * N + base + 128],
                                        mov[:, ic:ic + 1],
                                        start=(ic == 0),
                                        stop=(ic == NCH - 1),
                                    )
                        for p in duo:
                            s = st[p]
                            ps = s["ps"]
                            if t == 0:
                                pref = spool.tile([128, NCH], F32,
                                                  tag="pref")
                                nc.vector.tensor_add(pref[:], ps[:],
                                                     s["s0c"])
                                s["pref"] = pref
                            else:
                                nc.vector.tensor_add(s["pref"][:],
                                                     s["pref"][:], ps[:])
                            # clamp to {0,1}: exact in fp8, and doubles
                            # as the next step's moving operand
                            ct = spool.tile([128, NCH], F8, tag="ct")
                            nc.vector.tensor_scalar_min(ct[:],
                                                        s["pref"][:], 1.0)
                            s["ct"] = ct
                            nc.vector.scalar_tensor_tensor(
                                s["wcol"][:], s["ct"][:], coefs[t],
                                s["wcol"][:], op0=mult, op1=add)

                    for p in duo:
                        s = st[p]
                        dps = den_psum.tile([1, NCH], F32, tag="den")
                        nc.tensor.matmul(dps[:], ones128[:], s["ct"][:],
                                         start=True, stop=True)
                        den = spool.tile([1, 1], F32, tag="dens")
                        nc.vector.tensor_reduce(den[:], dps[:],
                                                axis=mybir.AxisListType.X,
                                                op=add)
                        # coverage count is an integer >= 1 unless the seed
                        # set is empty (w == 0 there, so any scale gives 0)
                        nc.vector.tensor_scalar_max(den[:], den[:], 0.5)
                        rec = spool.tile([1, 1], F32, tag="rec")
                        nc.vector.reciprocal(rec[:], den[:])
                        tmp = spool.tile([1, 1], F32, tag="tmp")
                        # fold mask and the ALPHA^4 rescale into one scalar
                        nc.vector.scalar_tensor_tensor(
                            tmp[:], rec[:], a4, mask_sb[:, p:p + 1],
                            op0=mult, op1=mult)
                        # broadcast the per-pair scalar to 128 partitions
                        # (activation scale must match the partition dim)
                        bcp = den_psum.tile([128, 1], F32, tag="den")
                        nc.tensor.matmul(bcp[:], onesrow[:], tmp[:],
                                         start=True, stop=True)
                        inv = spool.tile([128, 1], F32, tag="inv")
                        nc.scalar.copy(inv[:], bcp[:])
                        s["inv"] = inv
                        ubf = spool.tile([128, NCH], BF16, tag="ubf")
                        nc.vector.tensor_copy(ubf[:], s["wcol"][:])
                        s["ubf"] = ubf

                    # software pipeline: emit the PREVIOUS duo's fea
                    # here, so it fills the PE wait on this duo's DVE
                    # dependency chain (ubf/inv). F-stationary matmuls:
                    # output free size 1 -> near-zero PE time, and the
                    # column (transposed) layout falls out directly.
                    def emit_fea(pd, sd):
                        nfp = fea_psum.tile([128, DG], F32, tag="fea")
                        for g in range(DG):
                            for ic in range(NCH):
                                nc.tensor.matmul(
                                    nfp[:, g:g + 1],
                                    sd["F"][:, ic * D + g * 128:
                                            ic * D + g * 128 + 128],
                                    sd["ubf"][:, ic:ic + 1],
                                    start=(ic == 0),
                                    stop=(ic == NCH - 1),
                                )
                        nc.scalar.activation(nfT[:, pd::P_PER], nfp[:],
                                             relu, scale=sd["inv"][:])

                    for pd, sd in pending_fea:
                        emit_fea(pd, sd)
                    pending_fea = [(p, st[p]) for p in duo]

                    if pp == 6:
                        q_block()

                for pd, sd in pending_fea:
                    emit_fea(pd, sd)

            with (
                tc.tile_pool(name="mlpps", bufs=2, space="PSUM") as mps,
                tc.tile_pool(name="trps", bufs=1, space="PSUM") as tps,
            ):
                # nfT was filled per-pair inside the loop; all stages
                # use weight-stationary matmuls whose outputs are already
                # in transposed (column) layout. Biases fold in as one
                # extra bias-row x ones matmul per accumulation group so
                # each layer needs only ONE wide activation (fewer
                # serialized cross-engine hops on the critical tail).
                h1T = mpool.tile([128, DG * P_PER], BF16)
                hp1 = mps.tile([128, DG * P_PER], F32, tag="h")
                for go in range(DG):
                    cols = slice(go * P_PER, (go + 1) * P_PER)
                    for g in range(DG):
                        nc.tensor.matmul(
                            hp1[:, cols],
                            w1_sb[:, g * D + go * 128:g * D + go * 128 + 128],
                            nfT[:, g * P_PER:(g + 1) * P_PER],
                            start=(g == 0), stop=False)
                    nc.tensor.matmul(hp1[:, cols],
                                     b1row[0:1, go * 128:go * 128 + 128],
                                     ones16[:], start=False, stop=True)
                nc.scalar.activation(h1T[:], hp1[:], relu)

                h2T = mpool.tile([128, DG * P_PER], BF16)
                hp2 = mps.tile([128, DG * P_PER], F32, tag="h")
                for go in range(DG):
                    cols = slice(go * P_PER, (go + 1) * P_PER)
                    for g in range(DG):
                        nc.tensor.matmul(
                            hp2[:, cols],
                            w2_sb[:, g * D + go * 128:g * D + go * 128 + 128],
                            h1T[:, g * P_PER:(g + 1) * P_PER],
                            start=(g == 0), stop=False)
                    nc.tensor.matmul(hp2[:, cols],
                                     qb2r[0:1, go * 128:go * 128 + 128],
                                     ones16[:], start=False, stop=True)
                nc.scalar.activation(h2T[:], hp2[:], relu)

                yp = tps.tile([128, 1], F32, tag="tr")
                for g in range(DG):
                    nc.tensor.matmul(yp[0:P_PER, 0:1],
                                     h2T[:, g * P_PER:(g + 1) * P_PER],
                                     w3_sb[:, g:g + 1],
                                     start=(g == 0), stop=False)
                nc.tensor.matmul(yp[0:P_PER, 0:1], ones16[:], b3_sb[:],
                                 start=False, stop=True)
                ysb = mpool.tile([P_PER, 1], F32)
                nc.vector.tensor_copy(ysb[:], yp[0:P_PER, 0:1])
                nc.scalar.dma_start(y_out[:], ysb[:])

    nc.compile()
    return nc


def get_program():
    global _PROG
    if _PROG is None:
        _PROG = _build()
    return _PROG


def _pack4(bits):
    """Pack binary {0,1} float array [128, M] into 4-entries-per-byte u8:
    byte j = sum_k bits[:, 4j+k] << k (matches the DVE shift+mask unpack)."""
    b = bits.astype(np.uint8)
    return (b[:, 0::4] | (b[:, 1::4] << 1) | (b[:, 2::4] << 2)
            | (b[:, 3::4] << 3))


def _prep_core(core, query_fea, a_nei, vec_nei, fea_emb, nei_mask,
               W1, b1, W2, b2, W3, b3):
    b = (core * P_PER) // A
    a0 = (core * P_PER) % A
    a_loc = a_nei[b, a0:a0 + P_PER]
    f_loc = fea_emb[b, a0:a0 + P_PER]
    s_loc = vec_nei[b, a0:a0 + P_PER]
    return {
        "a_pre": _pack4(np.ascontiguousarray(
            a_loc.reshape(P_PER, NCH, 128, N).transpose(2, 0, 1, 3)
            .reshape(128, P_PER * NCH * N))),
        "f_pre": np.ascontiguousarray(
            f_loc.reshape(P_PER, NCH, 128, D).transpose(2, 0, 1, 3)
            .reshape(128, P_PER * NCH * D)).astype(BF16_NP),
        "s0_pre": np.ascontiguousarray(
            s_loc.reshape(P_PER, NCH, 128).transpose(2, 0, 1)
            .reshape(128, P_PER * NCH)).astype(F8_NP),
        "mask_pre": nei_mask[b, a0:a0 + P_PER, 0].reshape(1, P_PER)
        .astype(np.float32),
        "q_pre": query_fea[b].astype(BF16_NP),
        "w1_pre": np.ascontiguousarray(
            W1.reshape(DG, 128, D).transpose(1, 0, 2).reshape(128, DG * D))
        .astype(BF16_NP),
        "w2_pre": np.ascontiguousarray(
            W2.reshape(2 * DG, 128, D).transpose(1, 0, 2)
            .reshape(128, 2 * DG * D)).astype(BF16_NP),
        "w3_pre": np.ascontiguousarray(
            W3[:, 0].reshape(DG, 128).transpose(1, 0)).astype(BF16_NP),
        "b1_pre": b1.reshape(1, D).astype(BF16_NP),
        "b2_pre": b2.reshape(1, D).astype(np.float32),
        "b3_pre": b3.reshape(1, 1).astype(BF16_NP),
    }


_EXEC = None


def _make_exec():
    """Replicates bass2jax.run_bass_via_pjrt's multi-core path, but caches
    the jitted executable so repeated calls (and timing loops) skip
    recompilation."""
    global _EXEC
    if _EXEC is not None:
        return _EXEC
    import jax
    from jax.experimental.shard_map import shard_map
    from jax.sharding import Mesh, PartitionSpec

    from concourse import mybir as _mybir
    from concourse.bass2jax import (_bass_exec_p, install_neuronx_cc_hook,
                                    partition_id_tensor)

    nc = get_program()
    install_neuronx_cc_hook()
    partition_name = (nc.partition_id_tensor.name
                      if nc.partition_id_tensor else None)
    in_names, out_names, out_avals, zero_outs = [], [], [], []
    for alloc in nc.m.functions[0].allocations:
        if not isinstance(alloc, _mybir.MemoryLocationSet):
            continue
        name = alloc.memorylocations[0].name
        if alloc.kind == "ExternalInput":
            if name != partition_name:
                in_names.append(name)
        elif alloc.kind == "ExternalOutput":
            shape = tuple(alloc.tensor_shape)
            dtype = _mybir.dt.np(alloc.dtype)
            out_names.append(name)
            out_avals.append(jax.core.ShapedArray(shape, dtype))
            zero_outs.append(np.zeros(shape, dtype))
    n_params = len(in_names)
    all_in_names = list(in_names) + list(out_names)
    if partition_name is not None:
        all_in_names.append(partition_name)

    def _body(*args):
        operands = list(args)
        if partition_name is not None:
            operands.append(partition_id_tensor())
        outs = _bass_exec_p.bind(
            *operands,
            out_avals=tuple(out_avals),
            in_names=tuple(all_in_names),
            out_names=tuple(out_names),
            lowering_input_output_aliases=(),
            sim_require_finite=True,
            sim_require_nnan=True,
            nc=nc,
        )
        return tuple(outs)

    devices = jax.devices()[:NCORES]
    mesh = Mesh(np.asarray(devices), ("core",))
    n_outs = len(out_names)
    sharded = jax.jit(
        shard_map(_body, mesh=mesh,
                  in_specs=(PartitionSpec("core"),) * (n_params + n_outs),
                  out_specs=(PartitionSpec("core"),) * n_outs,
                  check_rep=False),
        keep_unused=True,
    )
    _EXEC = (sharded, in_names, out_names, out_avals, zero_outs, mesh)
    return _EXEC


def run_sharded(in_maps, reps=1):
    """Execute on 8 cores; returns (per-core results, [wall_ns per rep])."""
    import time as _time

    import jax

    sharded, in_names, out_names, out_avals, zero_outs, mesh = _make_exec()
    from jax.sharding import NamedSharding, PartitionSpec
    shard = NamedSharding(mesh, PartitionSpec("core"))
    concat_in = [
        jax.device_put(
            np.concatenate([np.asarray(in_maps[c][n])
                            for c in range(NCORES)], axis=0), shard)
        for n in in_names
    ]
    concat_zeros = [
        jax.device_put(
            np.zeros((NCORES * z.shape[0], *z.shape[1:]), z.dtype), shard)
        for z in zero_outs
    ]
    args = concat_in + concat_zeros
    jax.block_until_ready(args)
    out_arrs = None
    times = []
    for _ in range(max(1, reps)):
        t0 = _time.perf_counter()
        out_arrs = sharded(*args)
        jax.block_until_ready(out_arrs)
        times.append((_time.perf_counter() - t0) * 1e9)
    results = [
        {
            name: np.asarray(out_arrs[i]).reshape(
                NCORES, *out_avals[i].shape)[c]
            for i, name in enumerate(out_names)
        }
        for c in range(NCORES)
    ]
    return results, times


def kernel(query_fea, a_nei, vec_nei, fea_emb, nei_mask,
           W1, b1, W2, b2, W3, b3, trace=False, reps=1):
    global LAST_RESULT
    args = [np.asarray(x) for x in (query_fea, a_nei, vec_nei, fea_emb,
                                    nei_mask, W1, b1, W2, b2, W3, b3)]
    in_maps = [_prep_core(c, *args) for c in range(NCORES)]
    results, times = run_sharded(in_maps, reps=reps)
    LAST_RESULT = {"times_ns": times}
    ys = [results[c]["y"].reshape(P_PER) for c in range(NCORES)]
    return np.concatenate(ys).reshape(B, A, 1).astype(np.float32)
